# revision 26
# baseline (speedup 1.0000x reference)
"""Trainium2 Bass kernel for Gaussian-KDE logsumexp (nn_GaussianKernel).

out[n] = logsumexp_m( -0.5*||(y_n - x_m)/bw||^2 - Z ),  Z = D/2*log(2pi) + D*log(bw) + log(M)

On-device factorization (per query row n, data col m):
    A[n,m] = (y_n/bw^2) . x_m  +  c_m,       c_m = -||x_m||^2/(2 bw^2)   (host, fp64)
    out[n] = logsumexp_m A[n,m] + r_n,       r_n = -||y_n||^2/(2 bw^2) - Z (host, fp64)

y and x are quantized to bf16 once on the host; c_m / r_n are computed from the
quantized values, so the result is the exact logsumexp of slightly perturbed
points (error ~1e-3 relative; tolerance 2e-2).  c_m rides into PSUM as K=2
rank-2 bf16 matmuls (ones^T @ [c_hi; c_lo]); four bias matmuls at a time sit
in 4 distinct PE row-groups (tile_position) so they stream concurrently.

Sharding is 2-D: 4 query-shards x 2 data-halves over the 8 cores.  Core
c = 2*q + d handles query block q (512 rows = 4 M-tiles) against data half d
(1024 cols).  Each core returns per-tile partial (-rowmax, sum exp) pairs;
the host merges the two data-halves with an exact fp64 logsumexp merge and
adds r_n - so the device needs no ln() and no r/Z handling at all.

Per core: warmup matmuls run during the input-DMA wait to keep the PE clock
gate up; per M-tile the bias pass (start=True) + bf16 y.x pass (stop=True)
fill a [128,1024] PSUM tile, then DVE does one negated row-max (written
straight into the output pack), ACT does one exp with fused row-sum
accumulation (accumulator drained straight into the output pack).  The
[128,8] pack is rearranged onto 32 partitions with a DVE 32x32 stream
transpose so the output store is 4 DMAs of big descriptors instead of 128
4-byte read-modify-writes (~4us of completion latency).  The only ACT table
is exp, preloaded by a dummy activation at kernel start.
"""

import sys
from math import log, pi

import numpy as np
import ml_dtypes

sys.path.insert(0, "/opt/trn_rl_repo")

import concourse.bacc as bacc
import concourse.bass as bass
import concourse.mybir as mybir
import concourse.tile as tile
from concourse.bass_utils import run_bass_kernel_spmd

BW = 0.1
N_QUERY = 2048
N_DATA = 2048
DIM = 128
N_CORES = 8

N_QSHARDS = 4
N_DHALVES = 2
QSHARD = N_QUERY // N_QSHARDS      # 512 query rows per core
DHALF = N_DATA // N_DHALVES        # 1024 data cols per core
M_TILES = QSHARD // 128            # 4
NM = 512                           # matmul free-dim (one fp32 PSUM bank)
N_BANKS = DHALF // NM              # 2 banks per M-tile
N_WARMUP = 5                       # PE warmup matmuls during DMA wait

Z_CONST = 0.5 * DIM * log(2.0 * pi) + DIM * log(BW) + log(float(N_DATA))

_CACHE = {}


def _build_nc():
    f32 = mybir.dt.float32
    bf16 = mybir.dt.bfloat16
    fx = mybir.ActivationFunctionType
    nc = bacc.Bacc("TRN2", target_bir_lowering=False, debug=False)

    xt = nc.dram_tensor("xt", [DIM, DHALF], bf16, kind="ExternalInput")
    yt = nc.dram_tensor("yt", [DIM, QSHARD], bf16, kind="ExternalInput")
    # crow rows: (b0_hi, b1_hi, b0_hi, b1_hi) and (b0_lo, b1_lo, b0_lo, b1_lo)
    # land on SBUF partitions {0,32,64,96} and {1,33,65,97} respectively, so
    # row-group 32*i serves bank i%2 for two M-tiles at once.
    crow_hi_d = nc.dram_tensor("crow_hi", [4, NM], bf16, kind="ExternalInput")
    crow_lo_d = nc.dram_tensor("crow_lo", [4, NM], bf16, kind="ExternalInput")
    # out[16*tp + 4*i + c, r] = pack_tp[32*i + r, c]   (c = 2*u + kind,
    # tile t = 2*tp + u, query t*128 + 32*i + r)
    out = nc.dram_tensor("out", [32, 32], f32, kind="ExternalOutput")

    with tile.TileContext(nc) as tc:
        with (
            tc.tile_pool(name="io", bufs=1) as io,
            tc.tile_pool(name="psum", bufs=4, space=bass.MemorySpace.PSUM) as psum,
            tc.tile_pool(name="work", bufs=1) as work,
            tc.tile_pool(name="small", bufs=2) as small,
        ):
            # ---- constants first so the PE warmup can start ASAP
            junk2 = io.tile([2, NM], bf16, tag="junk2")
            nc.gpsimd.memset(junk2[:], 0.0)
            ones_sb = io.tile([128, 128], bf16, tag="ones")
            nc.gpsimd.memset(ones_sb[:], 1.0)

            # ---- dummy exp: pulls the ACT exp-table load off the critical path
            dmy = small.tile([1, 1], f32, tag="dmy")
            dmy2 = small.tile([1, 1], f32, tag="dmy2")
            nc.gpsimd.memset(dmy[:], 0.0)
            nc.scalar.activation(dmy2[:], dmy[:], fx.Exp)

            # ---- input DMAs (crow first: bias passes need only it; yt on the
            # scalar queue runs in parallel with the sync queue)
            crow = io.tile([128, NM], bf16, tag="crow")
            nc.sync.dma_start(crow[0:97:32, :], crow_hi_d[:])
            nc.sync.dma_start(crow[1:98:32, :], crow_lo_d[:])
            xt_sb = io.tile([DIM, DHALF], bf16, tag="xt")
            nc.sync.dma_start(xt_sb[:], xt[:])
            yt_sb = io.tile([DIM, QSHARD], bf16, tag="yt")
            nc.scalar.dma_start(yt_sb[:], yt[:])

            A = [psum.tile([128, DHALF], f32, tag="A", name=f"A{t}")
                 for t in range(M_TILES)]

            # ---- PE warmup: garbage matmuls (overwritten by the bias pass)
            for w in range(N_WARMUP):
                nc.tensor.matmul(A[0][:, :NM], ones_sb[0:2, :], junk2[:],
                                 start=True, stop=True)

            # per tile-pair: pack_p[:, 2u+k]: u = tile within pair, k = 0 for
            # -rowmax, 1 for sum exp  (separate packs so each pair's output
            # transpose + DMAs can fire as soon as its tiles finish)
            packs = [small.tile([128, 32], f32, tag="pack", name=f"pack{p}")
                     for p in range(M_TILES // 2)]
            esc = work.tile([128, DHALF], bf16, tag="esc")

            # ---- PE: per M-tile pair: 4 concurrent rank-2 bias matmuls
            # (tile t bank b in row-group 32*(2*(t%2)+b)), then y.x passes
            for tp in range(M_TILES // 2):
                pk = packs[tp]
                for t in (2 * tp, 2 * tp + 1):
                    for b in range(N_BANKS):
                        g = 32 * (2 * (t % 2) + b)
                        nc.tensor.matmul(A[t][:, b * NM:(b + 1) * NM],
                                         ones_sb[g:g + 2, :],
                                         crow[g:g + 2, :],
                                         start=True, stop=False,
                                         tile_position=(g, 0))
                for t in (2 * tp, 2 * tp + 1):
                    u = t % 2
                    for b in range(N_BANKS):
                        nc.tensor.matmul(A[t][:, b * NM:(b + 1) * NM],
                                         yt_sb[:, t * 128:(t + 1) * 128],
                                         xt_sb[:, b * NM:(b + 1) * NM],
                                         start=False, stop=True)
                    # DVE row-max and ACT exp+accum write the pack directly
                    nc.vector.tensor_reduce(pk[:, 2 * u:2 * u + 1], A[t][:],
                                            axis=mybir.AxisListType.X,
                                            op=mybir.AluOpType.max, negate=True)
                    nc.scalar.activation(esc[:], A[t][:], fx.Exp,
                                         bias=pk[:, 2 * u:2 * u + 1],
                                         scale=1.0,
                                         accum_out=pk[:, 2 * u + 1:2 * u + 2])

                # rearrange pair tp's [128,4] pack onto 16 partitions and
                # store: t32[32*i + c, r] = pack[32*i + r, c]; the first
                # pair's stores issue while the second pair is still on
                # PE/DVE/ACT.  out row 16*tp + 4*i + c <- t32[32*i + c].
                t32 = small.tile([128, 32], f32, tag="t32", name=f"t32_{tp}")
                nc.vector.transpose(t32[:], pk[:])
                for i in range(4):
                    eng = nc.sync if i % 2 == 0 else nc.scalar
                    eng.dma_start(out[16 * tp + 4 * i:16 * tp + 4 * i + 4, :],
                                  t32[32 * i:32 * i + 4, :])

    nc.compile()
    return nc


def make_in_maps(y, x):
    """Host-side prep: bf16 quantization + per-core shard layouts."""
    y = np.asarray(y, dtype=np.float32)
    x = np.asarray(x, dtype=np.float32)

    xq = x.astype(ml_dtypes.bfloat16)                       # (M, D) bf16
    xq64 = xq.astype(np.float64)
    c = (-0.5 / (BW * BW)) * np.sum(xq64 * xq64, axis=1)    # (M,) fp64
    c_hi = c.astype(ml_dtypes.bfloat16)
    c_lo = (c - c_hi.astype(np.float64)).astype(ml_dtypes.bfloat16)

    xts, crow_his, crow_los = [], [], []
    for d in range(N_DHALVES):
        sl = slice(d * DHALF, (d + 1) * DHALF)
        xts.append(np.ascontiguousarray(xq[sl].T))          # (D, DHALF)
        hi = c_hi[sl].reshape(N_BANKS, NM)                  # rows b0,b1
        lo = c_lo[sl].reshape(N_BANKS, NM)
        crow_his.append(np.ascontiguousarray(
            np.concatenate([hi, hi])))                      # (4, NM)
        crow_los.append(np.ascontiguousarray(np.concatenate([lo, lo])))

    in_maps = []
    for core in range(N_CORES):
        q, d = core // N_DHALVES, core % N_DHALVES
        ysh = y[q * QSHARD:(q + 1) * QSHARD]
        ytq = (ysh.astype(np.float64) / (BW * BW)).astype(ml_dtypes.bfloat16)
        in_maps.append({
            "xt": xts[d],
            "yt": np.ascontiguousarray(ytq.T),              # (D, QSHARD) bf16
            "crow_hi": crow_his[d],
            "crow_lo": crow_los[d],
        })
    return in_maps


def _unpack(res_out):
    """(32,32) device pack -> (nmax, s) arrays of shape (M_TILES, 128)."""
    # res_out[16*tp + 4*i + 2*u + k, r] = query (2*tp+u)*128 + 32*i + r
    o = res_out.reshape(2, 4, 4, 32).astype(np.float64)     # [tp, i, c, r]
    nmax = np.empty((M_TILES, 128))
    s = np.empty((M_TILES, 128))
    for t in range(M_TILES):
        tp, u = divmod(t, 2)
        nmax[t] = o[tp, :, 2 * u, :].reshape(128)           # i-major, then r
        s[t] = o[tp, :, 2 * u + 1, :].reshape(128)
    return nmax, s


def assemble_output(results, y):
    """Host-side fp64 merge of the two data-halves + r_n - Z correction."""
    y = np.asarray(y, dtype=np.float32)
    out = np.empty(N_QUERY, dtype=np.float64)
    for q in range(N_QSHARDS):
        parts = []
        for d in range(N_DHALVES):
            nmax, s = _unpack(results[q * N_DHALVES + d]["out"])
            parts.append((-nmax) + np.log(s))               # partial logsumexp
        L = np.logaddexp(parts[0], parts[1]).reshape(-1)    # (512,)
        ysh = y[q * QSHARD:(q + 1) * QSHARD]
        ytq = (ysh.astype(np.float64) / (BW * BW)).astype(ml_dtypes.bfloat16)
        yt64 = ytq.astype(np.float64)
        r = -0.5 * (BW * BW) * np.sum(yt64 * yt64, axis=1) - Z_CONST
        out[q * QSHARD:(q + 1) * QSHARD] = L + r
    return out.astype(np.float32)


def kernel(y, x):
    y = np.asarray(y, dtype=np.float32)
    x = np.asarray(x, dtype=np.float32)
    assert y.shape == (N_QUERY, DIM) and x.shape == (N_DATA, DIM)

    if "nc" not in _CACHE:
        _CACHE["nc"] = _build_nc()
    nc = _CACHE["nc"]

    in_maps = make_in_maps(y, x)
    res = run_bass_kernel_spmd(nc, in_maps, core_ids=list(range(N_CORES)))
    return assemble_output(res.results, y)


# revision 28
# speedup vs baseline: 1.0799x; 1.0799x over previous
"""Trainium2 Bass kernel for Gaussian-KDE logsumexp (nn_GaussianKernel).

out[n] = logsumexp_m( -0.5*||(y_n - x_m)/bw||^2 - Z ),  Z = D/2*log(2pi) + D*log(bw) + log(M)

On-device factorization (per query row n, data col m):
    A[n,m] = (y_n/bw^2) . x_m  +  c_m,       c_m = -||x_m||^2/(2 bw^2)   (host, fp64)
    out[n] = logsumexp_m A[n,m] + r_n,       r_n = -||y_n||^2/(2 bw^2) - Z (host, fp64)

y and x are quantized to bf16 once on the host; c_m / r_n are computed from the
quantized values, so the result is the exact logsumexp of slightly perturbed
points (error ~1e-3 relative; tolerance 2e-2).  c_m rides into PSUM as K=2
rank-2 bf16 matmuls (ones^T @ [c_hi; c_lo]); four bias matmuls at a time sit
in 4 distinct PE row-groups (tile_position) so they stream concurrently.

Sharding is 2-D: 4 query-shards x 2 data-halves over the 8 cores.  Core
c = 2*q + d handles query block q (512 rows = 4 M-tiles) against data half d
(1024 cols).  Each core returns per-tile partial (-rowmax, sum exp) pairs;
the host merges the two data-halves with an exact fp64 logsumexp merge and
adds r_n - so the device needs no ln() and no r/Z handling at all.

Per core: warmup matmuls run during the input-DMA wait to keep the PE clock
gate up; per M-tile the bias pass (start=True) + bf16 y.x pass (stop=True)
fill a [128,1024] PSUM tile, then DVE does one negated row-max (written
straight into the output pack), ACT does one exp with fused row-sum
accumulation (accumulator drained straight into the output pack).  The
[128,8] pack is rearranged onto 32 partitions with a DVE 32x32 stream
transpose so the output store is 4 DMAs of big descriptors instead of 128
4-byte read-modify-writes (~4us of completion latency).  The only ACT table
is exp, preloaded by a dummy activation at kernel start.
"""

import sys
from math import log, pi

import numpy as np
import ml_dtypes

sys.path.insert(0, "/opt/trn_rl_repo")

import concourse.bacc as bacc
import concourse.bass as bass
import concourse.mybir as mybir
import concourse.tile as tile
from concourse.bass_utils import run_bass_kernel_spmd

BW = 0.1
N_QUERY = 2048
N_DATA = 2048
DIM = 128
N_CORES = 8

N_QSHARDS = 4
N_DHALVES = 2
QSHARD = N_QUERY // N_QSHARDS      # 512 query rows per core
DHALF = N_DATA // N_DHALVES        # 1024 data cols per core
M_TILES = QSHARD // 128            # 4
NM = 512                           # matmul free-dim (one fp32 PSUM bank)
N_BANKS = DHALF // NM              # 2 banks per M-tile
N_WARMUP = 5                       # PE warmup matmuls during DMA wait

Z_CONST = 0.5 * DIM * log(2.0 * pi) + DIM * log(BW) + log(float(N_DATA))

_CACHE = {}


def _build_nc():
    f32 = mybir.dt.float32
    bf16 = mybir.dt.bfloat16
    fx = mybir.ActivationFunctionType
    nc = bacc.Bacc("TRN2", target_bir_lowering=False, debug=False)

    xt = nc.dram_tensor("xt", [DIM, DHALF], bf16, kind="ExternalInput")
    yt = nc.dram_tensor("yt", [DIM, QSHARD], bf16, kind="ExternalInput")
    # crow rows: (b0_hi, b1_hi, b0_hi, b1_hi) and (b0_lo, b1_lo, b0_lo, b1_lo)
    # land on SBUF partitions {0,32,64,96} and {1,33,65,97} respectively, so
    # row-group 32*i serves bank i%2 for two M-tiles at once.
    crow_hi_d = nc.dram_tensor("crow_hi", [4, NM], bf16, kind="ExternalInput")
    crow_lo_d = nc.dram_tensor("crow_lo", [4, NM], bf16, kind="ExternalInput")
    # out[16*tp + 4*i + c, r] = pack_tp[32*i + r, c]   (c = 2*u + kind,
    # tile t = 2*tp + u, query t*128 + 32*i + r)
    out = nc.dram_tensor("out", [32, 32], f32, kind="ExternalOutput")

    with tile.TileContext(nc) as tc:
        with (
            tc.tile_pool(name="io", bufs=1) as io,
            tc.tile_pool(name="psum", bufs=4, space=bass.MemorySpace.PSUM) as psum,
            tc.tile_pool(name="work", bufs=1) as work,
            tc.tile_pool(name="small", bufs=2) as small,
        ):
            # ---- constants first so the PE warmup can start ASAP
            junk2 = io.tile([2, NM], bf16, tag="junk2")
            nc.gpsimd.memset(junk2[:], 0.0)
            ones_sb = io.tile([128, 128], bf16, tag="ones")
            nc.gpsimd.memset(ones_sb[:], 1.0)

            # ---- dummy exp: pulls the ACT exp-table load off the critical path
            dmy = small.tile([1, 1], f32, tag="dmy")
            dmy2 = small.tile([1, 1], f32, tag="dmy2")
            nc.gpsimd.memset(dmy[:], 0.0)
            nc.scalar.activation(dmy2[:], dmy[:], fx.Exp)

            # ---- input DMAs (crow first: bias passes need only it; yt on the
            # scalar queue runs in parallel with the sync queue)
            crow = io.tile([128, NM], bf16, tag="crow")
            nc.sync.dma_start(crow[0:97:32, :], crow_hi_d[:])
            nc.sync.dma_start(crow[1:98:32, :], crow_lo_d[:])
            xt_sb = io.tile([DIM, DHALF], bf16, tag="xt")
            for h in range(N_BANKS):
                nc.sync.dma_start(xt_sb[:, h * NM:(h + 1) * NM],
                                  xt[:, h * NM:(h + 1) * NM])
            yt_sb = io.tile([DIM, QSHARD], bf16, tag="yt")
            nc.scalar.dma_start(yt_sb[:], yt[:])

            A = [psum.tile([128, DHALF], f32, tag="A", name=f"A{t}")
                 for t in range(M_TILES)]

            # ---- PE warmup: garbage matmuls (overwritten by the bias pass)
            for w in range(N_WARMUP):
                nc.tensor.matmul(A[0][:, :NM], ones_sb[0:2, :], junk2[:],
                                 start=True, stop=True)

            # per tile-pair: pack_p[:, 2u+k]: u = tile within pair, k = 0 for
            # -rowmax, 1 for sum exp  (separate packs so each pair's output
            # transpose + DMAs can fire as soon as its tiles finish)
            packs = [small.tile([128, 32], f32, tag="pack", name=f"pack{p}")
                     for p in range(M_TILES // 2)]
            esc = work.tile([128, DHALF], bf16, tag="esc")

            # ---- PE: per M-tile pair: 4 concurrent rank-2 bias matmuls
            # (tile t bank b in row-group 32*(2*(t%2)+b)), then y.x passes
            for tp in range(M_TILES // 2):
                pk = packs[tp]
                for t in (2 * tp, 2 * tp + 1):
                    for b in range(N_BANKS):
                        g = 32 * (2 * (t % 2) + b)
                        nc.tensor.matmul(A[t][:, b * NM:(b + 1) * NM],
                                         ones_sb[g:g + 2, :],
                                         crow[g:g + 2, :],
                                         start=True, stop=False,
                                         tile_position=(g, 0))
                for t in (2 * tp, 2 * tp + 1):
                    u = t % 2
                    for b in range(N_BANKS):
                        nc.tensor.matmul(A[t][:, b * NM:(b + 1) * NM],
                                         yt_sb[:, t * 128:(t + 1) * 128],
                                         xt_sb[:, b * NM:(b + 1) * NM],
                                         start=False, stop=True)
                    # DVE row-max and ACT exp+accum write the pack directly
                    nc.vector.tensor_reduce(pk[:, 2 * u:2 * u + 1], A[t][:],
                                            axis=mybir.AxisListType.X,
                                            op=mybir.AluOpType.max, negate=True)
                    nc.scalar.activation(esc[:], A[t][:], fx.Exp,
                                         bias=pk[:, 2 * u:2 * u + 1],
                                         scale=1.0,
                                         accum_out=pk[:, 2 * u + 1:2 * u + 2])

                # rearrange pair tp's [128,4] pack onto 16 partitions and
                # store: t32[32*i + c, r] = pack[32*i + r, c]; the first
                # pair's stores issue while the second pair is still on
                # PE/DVE/ACT.  out row 16*tp + 4*i + c <- t32[32*i + c].
                # pair 0's stores all go on the sync queue (issuing from the
                # scalar queue would block the remaining ACTIVATEs behind the
                # DMA-issue instructions); pair 1 splits across both queues
                # since the ACT engine is done by then.
                t32 = small.tile([128, 32], f32, tag="t32", name=f"t32_{tp}")
                nc.vector.transpose(t32[:], pk[:])
                for i in range(4):
                    eng = nc.scalar if (tp == 1 and i >= 2) else nc.sync
                    eng.dma_start(out[16 * tp + 4 * i:16 * tp + 4 * i + 4, :],
                                  t32[32 * i:32 * i + 4, :])

    nc.compile()
    return nc


def make_in_maps(y, x):
    """Host-side prep: bf16 quantization + per-core shard layouts."""
    y = np.asarray(y, dtype=np.float32)
    x = np.asarray(x, dtype=np.float32)

    xq = x.astype(ml_dtypes.bfloat16)                       # (M, D) bf16
    xq64 = xq.astype(np.float64)
    c = (-0.5 / (BW * BW)) * np.sum(xq64 * xq64, axis=1)    # (M,) fp64
    c_hi = c.astype(ml_dtypes.bfloat16)
    c_lo = (c - c_hi.astype(np.float64)).astype(ml_dtypes.bfloat16)

    xts, crow_his, crow_los = [], [], []
    for d in range(N_DHALVES):
        sl = slice(d * DHALF, (d + 1) * DHALF)
        xts.append(np.ascontiguousarray(xq[sl].T))          # (D, DHALF)
        hi = c_hi[sl].reshape(N_BANKS, NM)                  # rows b0,b1
        lo = c_lo[sl].reshape(N_BANKS, NM)
        crow_his.append(np.ascontiguousarray(
            np.concatenate([hi, hi])))                      # (4, NM)
        crow_los.append(np.ascontiguousarray(np.concatenate([lo, lo])))

    in_maps = []
    for core in range(N_CORES):
        q, d = core // N_DHALVES, core % N_DHALVES
        ysh = y[q * QSHARD:(q + 1) * QSHARD]
        ytq = (ysh.astype(np.float64) / (BW * BW)).astype(ml_dtypes.bfloat16)
        in_maps.append({
            "xt": xts[d],
            "yt": np.ascontiguousarray(ytq.T),              # (D, QSHARD) bf16
            "crow_hi": crow_his[d],
            "crow_lo": crow_los[d],
        })
    return in_maps


def _unpack(res_out):
    """(32,32) device pack -> (nmax, s) arrays of shape (M_TILES, 128)."""
    # res_out[16*tp + 4*i + 2*u + k, r] = query (2*tp+u)*128 + 32*i + r
    o = res_out.reshape(2, 4, 4, 32).astype(np.float64)     # [tp, i, c, r]
    nmax = np.empty((M_TILES, 128))
    s = np.empty((M_TILES, 128))
    for t in range(M_TILES):
        tp, u = divmod(t, 2)
        nmax[t] = o[tp, :, 2 * u, :].reshape(128)           # i-major, then r
        s[t] = o[tp, :, 2 * u + 1, :].reshape(128)
    return nmax, s


def assemble_output(results, y):
    """Host-side fp64 merge of the two data-halves + r_n - Z correction."""
    y = np.asarray(y, dtype=np.float32)
    out = np.empty(N_QUERY, dtype=np.float64)
    for q in range(N_QSHARDS):
        parts = []
        for d in range(N_DHALVES):
            nmax, s = _unpack(results[q * N_DHALVES + d]["out"])
            parts.append((-nmax) + np.log(s))               # partial logsumexp
        L = np.logaddexp(parts[0], parts[1]).reshape(-1)    # (512,)
        ysh = y[q * QSHARD:(q + 1) * QSHARD]
        ytq = (ysh.astype(np.float64) / (BW * BW)).astype(ml_dtypes.bfloat16)
        yt64 = ytq.astype(np.float64)
        r = -0.5 * (BW * BW) * np.sum(yt64 * yt64, axis=1) - Z_CONST
        out[q * QSHARD:(q + 1) * QSHARD] = L + r
    return out.astype(np.float32)


def kernel(y, x):
    y = np.asarray(y, dtype=np.float32)
    x = np.asarray(x, dtype=np.float32)
    assert y.shape == (N_QUERY, DIM) and x.shape == (N_DATA, DIM)

    if "nc" not in _CACHE:
        _CACHE["nc"] = _build_nc()
    nc = _CACHE["nc"]

    in_maps = make_in_maps(y, x)
    res = run_bass_kernel_spmd(nc, in_maps, core_ids=list(range(N_CORES)))
    return assemble_output(res.results, y)


# revision 31
# speedup vs baseline: 1.1437x; 1.0591x over previous
"""Trainium2 Bass kernel for Gaussian-KDE logsumexp (nn_GaussianKernel).

out[n] = logsumexp_m( -0.5*||(y_n - x_m)/bw||^2 - Z ),  Z = D/2*log(2pi) + D*log(bw) + log(M)

On-device factorization (per query row n, data col m):
    A[n,m] = (y_n/bw^2) . x_m  +  c_m,       c_m = -||x_m||^2/(2 bw^2)   (host, fp64)
    out[n] = logsumexp_m A[n,m] + r_n,       r_n = -||y_n||^2/(2 bw^2) - Z (host, fp64)

y and x are quantized to bf16 once on the host; c_m / r_n are computed from the
quantized values, so the result is the exact logsumexp of slightly perturbed
points (error ~1e-3 relative; tolerance 2e-2).  c_m rides into PSUM as K=2
rank-2 bf16 matmuls (ones^T @ [c_hi; c_lo]); four bias matmuls at a time sit
in 4 distinct PE row-groups (tile_position) so they stream concurrently.

Sharding is 2-D: 4 query-shards x 2 data-halves over the 8 cores.  Core
c = 2*q + d handles query block q (512 rows = 4 M-tiles) against data half d
(1024 cols).  Each core returns per-tile partial (-rowmax, sum exp) pairs;
the host merges the two data-halves with an exact fp64 logsumexp merge and
adds r_n - so the device needs no ln() and no r/Z handling at all.

Per core: warmup matmuls run during the input-DMA wait to keep the PE clock
gate up; per M-tile the bias pass (start=True) + bf16 y.x pass (stop=True)
fill a [128,1024] PSUM tile, then DVE does one negated row-max (written
straight into the output pack), ACT does one exp with fused row-sum
accumulation (accumulator drained straight into the output pack).  The
[128,8] pack is rearranged onto 32 partitions with a DVE 32x32 stream
transpose so the output store is 4 DMAs of big descriptors instead of 128
4-byte read-modify-writes (~4us of completion latency).  The only ACT table
is exp, preloaded by a dummy activation at kernel start.
"""

import sys
from math import log, pi

import numpy as np
import ml_dtypes

sys.path.insert(0, "/opt/trn_rl_repo")

import concourse.bacc as bacc
import concourse.bass as bass
import concourse.mybir as mybir
import concourse.tile as tile
from concourse.bass_utils import run_bass_kernel_spmd

BW = 0.1
N_QUERY = 2048
N_DATA = 2048
DIM = 128
N_CORES = 8

N_QSHARDS = 4
N_DHALVES = 2
QSHARD = N_QUERY // N_QSHARDS      # 512 query rows per core
DHALF = N_DATA // N_DHALVES        # 1024 data cols per core
M_TILES = QSHARD // 128            # 4
NM = 512                           # matmul free-dim (one fp32 PSUM bank)
N_BANKS = DHALF // NM              # 2 banks per M-tile
N_WARMUP = 5                       # PE warmup matmuls during DMA wait

Z_CONST = 0.5 * DIM * log(2.0 * pi) + DIM * log(BW) + log(float(N_DATA))

_CACHE = {}


def _build_nc():
    f32 = mybir.dt.float32
    bf16 = mybir.dt.bfloat16
    fx = mybir.ActivationFunctionType
    nc = bacc.Bacc("TRN2", target_bir_lowering=False, debug=False)

    xt = nc.dram_tensor("xt", [DIM, DHALF], bf16, kind="ExternalInput")
    yt = nc.dram_tensor("yt", [DIM, QSHARD], bf16, kind="ExternalInput")
    # crow rows: (b0_hi, b1_hi, b0_hi, b1_hi) and (b0_lo, b1_lo, b0_lo, b1_lo)
    # land on SBUF partitions {0,32,64,96} and {1,33,65,97} respectively, so
    # row-group 32*i serves bank i%2 for two M-tiles at once.
    crow_hi_d = nc.dram_tensor("crow_hi", [4, NM], bf16, kind="ExternalInput")
    crow_lo_d = nc.dram_tensor("crow_lo", [4, NM], bf16, kind="ExternalInput")
    # out[128*tp + 32*i + c, r] = pack_tp[32*i + r, c]   (c = 2*u + kind,
    # tile t = 2*tp + u, query t*128 + 32*i + r; c >= 4 is garbage filler)
    out = nc.dram_tensor("out", [256, 32], f32, kind="ExternalOutput")

    with tile.TileContext(nc) as tc:
        with (
            tc.tile_pool(name="io", bufs=1) as io,
            tc.tile_pool(name="psum", bufs=4, space=bass.MemorySpace.PSUM) as psum,
            tc.tile_pool(name="work", bufs=1) as work,
            tc.tile_pool(name="small", bufs=2) as small,
        ):
            # ---- constants first so the PE warmup can start ASAP
            junk2 = io.tile([2, NM], bf16, tag="junk2")
            nc.gpsimd.memset(junk2[:], 0.0)
            ones_sb = io.tile([128, 128], bf16, tag="ones")
            nc.gpsimd.memset(ones_sb[:], 1.0)

            # ---- dummy exp: pulls the ACT exp-table load off the critical path
            dmy = small.tile([1, 1], f32, tag="dmy")
            dmy2 = small.tile([1, 1], f32, tag="dmy2")
            nc.gpsimd.memset(dmy[:], 0.0)
            nc.scalar.activation(dmy2[:], dmy[:], fx.Exp)

            # ---- input DMAs (crow first: bias passes need only it; yt on the
            # scalar queue runs in parallel with the sync queue)
            crow = io.tile([128, NM], bf16, tag="crow")
            nc.sync.dma_start(crow[0:97:32, :], crow_hi_d[:])
            nc.sync.dma_start(crow[1:98:32, :], crow_lo_d[:])
            xt_sb = io.tile([DIM, DHALF], bf16, tag="xt")
            for h in range(N_BANKS):
                nc.sync.dma_start(xt_sb[:, h * NM:(h + 1) * NM],
                                  xt[:, h * NM:(h + 1) * NM])
            yt_sb = io.tile([DIM, QSHARD], bf16, tag="yt")
            nc.scalar.dma_start(yt_sb[:], yt[:])

            A = [psum.tile([128, DHALF], f32, tag="A", name=f"A{t}")
                 for t in range(M_TILES)]

            # ---- PE warmup: garbage matmuls (overwritten by the bias pass)
            for w in range(N_WARMUP):
                nc.tensor.matmul(A[0][:, :NM], ones_sb[0:2, :], junk2[:],
                                 start=True, stop=True)

            # per tile-pair: pack_p[:, 2u+k]: u = tile within pair, k = 0 for
            # -rowmax, 1 for sum exp  (separate packs so each pair's output
            # transpose + DMAs can fire as soon as its tiles finish)
            packs = [small.tile([128, 32], f32, tag="pack", name=f"pack{p}")
                     for p in range(M_TILES // 2)]
            esc = work.tile([128, DHALF], bf16, tag="esc")

            # ---- PE: per M-tile pair: 4 concurrent rank-2 bias matmuls
            # (tile t bank b in row-group 32*(2*(t%2)+b)), then y.x passes
            for tp in range(M_TILES // 2):
                pk = packs[tp]
                for t in (2 * tp, 2 * tp + 1):
                    for b in range(N_BANKS):
                        g = 32 * (2 * (t % 2) + b)
                        nc.tensor.matmul(A[t][:, b * NM:(b + 1) * NM],
                                         ones_sb[g:g + 2, :],
                                         crow[g:g + 2, :],
                                         start=True, stop=False,
                                         tile_position=(g, 0))
                for t in (2 * tp, 2 * tp + 1):
                    u = t % 2
                    for b in range(N_BANKS):
                        nc.tensor.matmul(A[t][:, b * NM:(b + 1) * NM],
                                         yt_sb[:, t * 128:(t + 1) * 128],
                                         xt_sb[:, b * NM:(b + 1) * NM],
                                         start=False, stop=True)
                    # DVE row-max and ACT exp+accum write the pack directly
                    nc.vector.tensor_reduce(pk[:, 2 * u:2 * u + 1], A[t][:],
                                            axis=mybir.AxisListType.X,
                                            op=mybir.AluOpType.max, negate=True)
                    nc.scalar.activation(esc[:], A[t][:], fx.Exp,
                                         bias=pk[:, 2 * u:2 * u + 1],
                                         scale=1.0,
                                         accum_out=pk[:, 2 * u + 1:2 * u + 2])

                # rearrange pair tp's [128,4] pack onto 16 partitions and
                # store: t32[32*i + c, r] = pack[32*i + r, c]; the first
                # pair's stores issue while the second pair is still on
                # PE/DVE/ACT.  out row 16*tp + 4*i + c <- t32[32*i + c].
                # one whole-tile store per pair on the sync queue (scalar-
                # queue DMA issues would block the remaining ACTIVATEs);
                # pair 0's store runs while pair 1 is still computing.
                t32 = small.tile([128, 32], f32, tag="t32", name=f"t32_{tp}")
                nc.vector.transpose(t32[:], pk[:])
                nc.sync.dma_start(out[128 * tp:128 * (tp + 1), :], t32[:])

    nc.compile()
    return nc


def make_in_maps(y, x):
    """Host-side prep: bf16 quantization + per-core shard layouts."""
    y = np.asarray(y, dtype=np.float32)
    x = np.asarray(x, dtype=np.float32)

    xq = x.astype(ml_dtypes.bfloat16)                       # (M, D) bf16
    xq64 = xq.astype(np.float64)
    c = (-0.5 / (BW * BW)) * np.sum(xq64 * xq64, axis=1)    # (M,) fp64
    c_hi = c.astype(ml_dtypes.bfloat16)
    c_lo = (c - c_hi.astype(np.float64)).astype(ml_dtypes.bfloat16)

    xts, crow_his, crow_los = [], [], []
    for d in range(N_DHALVES):
        sl = slice(d * DHALF, (d + 1) * DHALF)
        xts.append(np.ascontiguousarray(xq[sl].T))          # (D, DHALF)
        hi = c_hi[sl].reshape(N_BANKS, NM)                  # rows b0,b1
        lo = c_lo[sl].reshape(N_BANKS, NM)
        crow_his.append(np.ascontiguousarray(
            np.concatenate([hi, hi])))                      # (4, NM)
        crow_los.append(np.ascontiguousarray(np.concatenate([lo, lo])))

    in_maps = []
    for core in range(N_CORES):
        q, d = core // N_DHALVES, core % N_DHALVES
        ysh = y[q * QSHARD:(q + 1) * QSHARD]
        ytq = (ysh.astype(np.float64) / (BW * BW)).astype(ml_dtypes.bfloat16)
        in_maps.append({
            "xt": xts[d],
            "yt": np.ascontiguousarray(ytq.T),              # (D, QSHARD) bf16
            "crow_hi": crow_his[d],
            "crow_lo": crow_los[d],
        })
    return in_maps


def _unpack(res_out):
    """(256,32) device pack -> (nmax, s) arrays of shape (M_TILES, 128)."""
    # res_out[128*tp + 32*i + 2*u + k, r] = query (2*tp+u)*128 + 32*i + r
    o = res_out.reshape(2, 4, 32, 32).astype(np.float64)    # [tp, i, c, r]
    nmax = np.empty((M_TILES, 128))
    s = np.empty((M_TILES, 128))
    for t in range(M_TILES):
        tp, u = divmod(t, 2)
        nmax[t] = o[tp, :, 2 * u, :].reshape(128)           # i-major, then r
        s[t] = o[tp, :, 2 * u + 1, :].reshape(128)
    return nmax, s


def assemble_output(results, y):
    """Host-side fp64 merge of the two data-halves + r_n - Z correction."""
    y = np.asarray(y, dtype=np.float32)
    out = np.empty(N_QUERY, dtype=np.float64)
    for q in range(N_QSHARDS):
        parts = []
        for d in range(N_DHALVES):
            nmax, s = _unpack(results[q * N_DHALVES + d]["out"])
            parts.append((-nmax) + np.log(s))               # partial logsumexp
        L = np.logaddexp(parts[0], parts[1]).reshape(-1)    # (512,)
        ysh = y[q * QSHARD:(q + 1) * QSHARD]
        ytq = (ysh.astype(np.float64) / (BW * BW)).astype(ml_dtypes.bfloat16)
        yt64 = ytq.astype(np.float64)
        r = -0.5 * (BW * BW) * np.sum(yt64 * yt64, axis=1) - Z_CONST
        out[q * QSHARD:(q + 1) * QSHARD] = L + r
    return out.astype(np.float32)


def kernel(y, x):
    y = np.asarray(y, dtype=np.float32)
    x = np.asarray(x, dtype=np.float32)
    assert y.shape == (N_QUERY, DIM) and x.shape == (N_DATA, DIM)

    if "nc" not in _CACHE:
        _CACHE["nc"] = _build_nc()
    nc = _CACHE["nc"]

    in_maps = make_in_maps(y, x)
    res = run_bass_kernel_spmd(nc, in_maps, core_ids=list(range(N_CORES)))
    return assemble_output(res.results, y)
